# revision 20
# baseline (speedup 1.0000x reference)
"""StyleGAN2 up-2x blur (upfirdn2d, up=2, pad=(2,1), 4x4 kernel) on 8 trn2 cores.

x: (4, 64, 256, 256) f32, kernel: (4, 4) f32 -> out: (4, 64, 511, 511) f32.

Polyphase decomposition: out[2r+s, 2c+t] is a 2x2-tap conv of x with weights
from the flipped kernel w = kernel[::-1, ::-1]:
  s=0 -> vertical taps (w[0,kx] @ r-1, w[2,kx] @ r); s=1 -> (w[1,kx] @ r, w[3,kx] @ r+1)
  t=0 -> horizontal taps kx in {0 (c-1), 2 (c)};    t=1 -> kx in {1 (c), 3 (c+1)}

Sharding: pure data parallel over the 256 (N*C) planes, 32 planes/core.

Numerics: the 2e-2 tolerance allows bf16 end-to-end - x is rounded to bf16
host-side (the [1,3,3,1] filter is exactly bf16-representable; a w_lo
correction path exists for general kernels), matmuls accumulate in fp32
PSUM, the output is stored as bf16 and upconverted on the host (rel err
~5e-3).  This halves both load and store HBM traffic.

Perf model (hard-won from traces):
- SWDGE stores are paced by Q7 descriptor emission (~10ns/desc, serial) and
  EACH SWDGE OP DRAINS ON A SINGLE SDMA ENGINE (~27 GB/s with 32KB packets).
  So the output is stored as an SBUF-LINEAR dump (the host unscrambles the
  layout for free): 16KB contiguous descriptors, 512 of them total, split
  into 64 ops of [16 partitions, 16KB] so consecutive ops round-robin onto
  all 16 engines.  Aggregate store rate then sits at the HBM write ceiling
  (~215 GB/s).  HWDGE stores are 10x worse (dispatch serializes ~10.5us/op
  on the issuing engine); scatter-to-NCHW layouts cost 8128 descriptors of
  Q7 emission; 4-d APs hit a slower software emission path - all dead ends.
- The whole output (131KB/partition) is buffered in SBUF (mpool bufs=8), so
  TensorE never stalls on stores: PE micro-idle would re-throttle the HAM
  clock (k=8 -> k=4) and double matmul time.
- The vertical 2-tap combine runs on TensorE as banded [128,128] stationary
  matmuls; horizontal taps are column-shifted moving operands accumulating
  into the same PSUM bank; 2 planes packed per matmul (free = 512).
- Output rows 254/255/256 straddle the two 128-row input chunks; they are
  computed for all 32 planes at once with gather stationaries over a
  [96, 256] tile holding plane rows 126/127/128 (4 matmuls total).
- Input is host-packed as [pair, r, c, g, w] so each load is one 256KB
  HWDGE op with 2KB contiguous per-partition chunks (HWDGE ops do spread
  across all 16 engines), and chunk-c moving slices stay contiguous.
"""

import os
import numpy as np
import ml_dtypes

_BF = ml_dtypes.bfloat16
_NCORES = 8
_PL = 32            # planes per core
_NPAIR = _PL // 2   # plane pairs per core
_MEGA = 4           # plane pairs per mega store tile
_NMEGA = _NPAIR // _MEGA
_H = 256
_W = 256
_OW = 511
_OP = 512           # padded output row pitch (host crops col 511)

_cache = {}
last_exec_ns = None
last_results = None


def _build(wlo_nz: bool):
    from contextlib import ExitStack
    import concourse.mybir as mybir
    import concourse.tile as tile
    from concourse import bacc

    BF = mybir.dt.bfloat16
    F32 = mybir.dt.float32

    nc = bacc.Bacc("TRN2", target_bir_lowering=False, debug=False)
    # input planes, host-packed: [pair, r, c(row chunk), g(plane in pair), w]
    # with plane row = c*128 + r
    xl = nc.dram_tensor("xl", [_NPAIR, 128, 2, 2, _W], BF, kind="ExternalInput").ap()
    sth = nc.dram_tensor("sth", [128, 12, 128], BF, kind="ExternalInput").ap()
    sgh = nc.dram_tensor("sgh", [128, 4, 128], BF, kind="ExternalInput").ap()
    if wlo_nz:
        stl = nc.dram_tensor("stl", [128, 12, 128], BF, kind="ExternalInput").ap()
        sgl = nc.dram_tensor("sgl", [128, 4, 128], BF, kind="ExternalInput").ap()
    # device output is SBUF-linear ("scrambled"); the host unscrambles.
    # out[mega, p, pl, cb, slot, w]: plane mega*2*_MEGA+pl, row-chunk cb,
    # out row (0 if cb==0 else 257) + 2p + slot, col w (pitch 512, 511=pad)
    out = nc.dram_tensor("out", [_NMEGA, 127, 2 * _MEGA, 2, 2, _OP],
                         BF, kind="ExternalOutput").ap()
    # seam rows: outs[3*plane + bi, w] = out row 254+bi of plane
    outs = nc.dram_tensor("outs", [96, _OP], BF, kind="ExternalOutput").ap()

    ncopy = 0  # alternate evacuation copies between VectorE and ScalarE

    with tile.TileContext(nc) as tc, ExitStack() as ctx:
        cpool = ctx.enter_context(tc.tile_pool(name="const", bufs=1))
        tpool = ctx.enter_context(tc.tile_pool(name="tin", bufs=4))
        epool = ctx.enter_context(tc.tile_pool(name="edge", bufs=1))
        mpool = ctx.enter_context(tc.tile_pool(name="mega", bufs=_NMEGA))
        bpool = ctx.enter_context(tc.tile_pool(name="bnd", bufs=1))
        ppool = ctx.enter_context(tc.tile_pool(name="ps", bufs=8, space="PSUM"))

        sth_t = cpool.tile([128, 12, 128], BF)
        nc.sync.dma_start(out=sth_t[:, :, :], in_=sth)
        sgh_t = cpool.tile([128, 4, 128], BF)
        nc.sync.dma_start(out=sgh_t[:, :, :], in_=sgh)
        if wlo_nz:
            stl_t = cpool.tile([128, 12, 128], BF)
            nc.sync.dma_start(out=stl_t[:, :, :], in_=stl)
            sgl_t = cpool.tile([128, 4, 128], BF)
            nc.sync.dma_start(out=sgl_t[:, :, :], in_=sgl)

        def copy_out(dst, src):
            nonlocal ncopy
            if ncopy % 2 == 0:
                nc.vector.tensor_copy(out=dst, in_=src)
            else:
                nc.scalar.copy(out=dst, in_=src)
            ncopy += 1

        # ---- seam rows oy=254 (s0,r=127: x[126],x[127]), oy=255 (s1,r=127:
        # ---- x[127],x[128]), oy=256 (s0,r=128: x[127],x[128]) for all 32
        # planes at once: e holds plane rows 126/127/128 at partition
        # 32*rk + 16*g + pair; gather stationaries sgh[:, kx, :] map them to
        # psum partition q = 3*plane + bi with the vertical taps folded in.
        # Emitted after mega 0 so the main pipeline starts immediately.
        def emit_seam():
            e = epool.tile([96, _W], BF)
            for rk, row in enumerate((126, 127, 128)):
                rr, cc = row % 128, row // 128
                for g in (0, 1):
                    nc.sync.dma_start(out=e[32 * rk + 16 * g:32 * rk + 16 * g + 16, :],
                                      in_=xl[:, rr, cc, g, :])

            bt = bpool.tile([96, _OP], BF)
            pb = ppool.tile([96, 2, 256], F32, tag="ps")
            mms = []
            for t_, kx, mv, pc in ((0, 2, (0, 256), (0, 256)), (0, 0, (0, 255), (1, 256)),
                                   (1, 1, (0, 256), (0, 256)), (1, 3, (1, 256), (0, 255))):
                mms.append((t_, kx, mv, pc, "h"))
                if wlo_nz:
                    mms.append((t_, kx, mv, pc, "l"))
            for i, (t_, kx, mv, pc, wp) in enumerate(mms):
                sg = sgh_t if wp == "h" else sgl_t
                nc.tensor.matmul(
                    pb[:, t_, pc[0]:pc[1]], sg[0:96, kx, 0:96], e[:, mv[0]:mv[1]],
                    start=(i == 0), stop=(i == len(mms) - 1))
            copy_out(bt[:, 0:_OP:2], pb[:, 0, :])
            copy_out(bt[:, 1:_OP:2], pb[:, 1, :])
            nc.gpsimd.dma_start(out=outs, in_=bt[:, :])

        # ---- main body: 4 megas x 4 pairs x 2 row-chunks
        # stationary groups: 0 = s0/chunkA (rows 0..126), 1 = s0/chunkB, 2 = s1
        for mega in range(_NMEGA):
            # mega assembly tile: [p, plane_in_mega, chunk, rowpair slot, col]
            m = mpool.tile([128, 2 * _MEGA, 2, 2, _OP], BF, tag="m")
            for pp in range(_MEGA):
                pair = mega * _MEGA + pp
                t = tpool.tile([128, 2, 2, _W], BF, tag="tin")
                nc.sync.dma_start(
                    out=t[:, :, :, :].rearrange("r c g w -> r (c g w)"),
                    in_=xl[pair].rearrange("r c g w -> r (c g w)"))
                for chunk in (0, 1):
                    ig0 = 0 if chunk == 0 else 1
                    # chunk A row-pair layout: [i,0]=s0A[i] (oy 2i),
                    # [i,1]=s1A[i] (oy 2i+1); chunk B: [i,0]=s1B[i]
                    # (oy 257+2i), [i,1]=s0B[i] (oy 258+2i)
                    rows = ((0, 1) if chunk == 0 else (1, 0))  # s feeding (slot0, slot1)
                    # 2-plane-packed moving/psum as FLAT 1-d APs (flat index
                    # = 256*plane + col): a 2-d free AP costs the PE a ~200
                    # cycle bubble per outer step (measured 379ns vs 213ns
                    # for N=512).  The plane-crossing element of the
                    # column-shifted kx=3 tap lands in psum col 255 = the
                    # cropped pad column, so it's harmless; kx=0 (shift the
                    # other way) would pollute plane 1's col 0, so it keeps
                    # the 2-d AP.
                    tv = t[:, chunk, :, :].rearrange("r g w -> r (g w)")
                    for s, ig in ((0, ig0), (1, 2)):
                        for t_, kxmv in ((0, ((2, "1d", (0, 512), (0, 512)), (0, "2d", (0, 255), (1, 256)))),
                                         (1, ((1, "1d", (0, 512), (0, 512)), (3, "1d", (1, 512), (0, 511))))):
                            pt = ppool.tile([128, 2, 256], F32, tag="ps")
                            pv = pt[:, :, :].rearrange("p g w -> p (g w)")
                            mms = []
                            for kx, md, mv, pc in kxmv:
                                mms.append((ig * 4 + kx, md, mv, pc, "h"))
                                if wlo_nz:
                                    mms.append((ig * 4 + kx, md, mv, pc, "l"))
                            for i, (j, md, mv, pc, wp) in enumerate(mms):
                                st_ = sth_t if wp == "h" else stl_t
                                if md == "1d":
                                    nc.tensor.matmul(
                                        pv[:, pc[0]:pc[1]], st_[:, j, :],
                                        tv[:, mv[0]:mv[1]],
                                        start=(i == 0), stop=(i == len(mms) - 1))
                                else:
                                    nc.tensor.matmul(
                                        pt[:, :, pc[0]:pc[1]], st_[:, j, :],
                                        t[:, chunk, :, mv[0]:mv[1]],
                                        start=(i == 0), stop=(i == len(mms) - 1))
                            # drain this psum group immediately (overlaps with
                            # the next group's matmuls), casting f32 -> bf16;
                            # both planes in one op (free dims [2, 256])
                            slot = rows.index(s)
                            dst = m[0:127, 2 * pp:2 * pp + 2, chunk, slot, :]
                            off = 0 if t_ == 0 else 1
                            copy_out(dst[:, :, off:_OP:2], pt[0:127, :, :])

            # store: 16 SWDGE ops per mega, SBUF-linear dump ([8 partitions,
            # 32KB descs]); one op drains on one SDMA engine at ~27GB/s with
            # 32KB packets, so 16 consecutive ops stripe all 16 engines
            for e_ in range(16):
                p0, p1 = 8 * e_, min(8 * e_ + 8, 127)
                nc.gpsimd.dma_start(
                    out=out[mega, p0:p1].rearrange("p a b c w -> p (a b c w)"),
                    in_=m[p0:p1, :, :, :, :].rearrange("p a b c w -> p (a b c w)"))
            if mega == 0:
                emit_seam()

    nc.compile()
    return nc


def _host_arrays(w):
    w = np.asarray(w, np.float32)
    w_hi = w.astype(_BF).astype(np.float32)
    w_lo = w - w_hi
    wlo_nz = bool(np.any(w_lo != 0))

    def build_st(wv):
        st = np.zeros((3, 4, 128, 128), np.float32)
        i6 = np.arange(126)
        i7 = np.arange(127)
        for kx in range(4):
            st[0, kx][i6, i6 + 1] = wv[0, kx]        # s0A subdiag, out rows 1..126
            st[0, kx][i7, i7] += wv[2, kx]           # s0A diag, out rows 0..126
            st[1, kx][i7, i7] = wv[0, kx]            # s0B diag
            st[1, kx][i7 + 1, i7] = wv[2, kx]        # s0B sub
            st[2, kx][i7, i7] = wv[1, kx]            # s1 diag
            st[2, kx][i7 + 1, i7] = wv[3, kx]        # s1 sub
        # [g,kx,p,i] -> [p, g*4+kx, i]
        return np.ascontiguousarray(
            st.reshape(12, 128, 128).transpose(1, 0, 2)).astype(_BF)

    def build_sg(wv):
        # seam gather: partition p = 32*rk + 16*g + pair holds plane row
        # (126,127,128)[rk] of plane 2*pair+g; output q = 3*plane + bi
        # (bi: seam row 254+bi); vertical taps per bi: (rk, ky) pairs
        taps = {0: ((0, 0), (1, 2)), 1: ((1, 1), (2, 3)), 2: ((1, 0), (2, 2))}
        sg = np.zeros((4, 128, 128), np.float32)
        for kx in range(4):
            for pair in range(16):
                for g in (0, 1):
                    plane = 2 * pair + g
                    for bi, tl in taps.items():
                        for rk, ky in tl:
                            sg[kx][32 * rk + 16 * g + pair, 3 * plane + bi] += wv[ky, kx]
        return np.ascontiguousarray(sg.transpose(1, 0, 2)).astype(_BF)  # [p, kx, q]

    arrs = {"sth": build_st(w_hi), "sgh": build_sg(w_hi)}
    if wlo_nz:
        wlo_b = w_lo.astype(_BF).astype(np.float32)
        arrs["stl"] = build_st(wlo_b)
        arrs["sgl"] = build_sg(wlo_b)
    return wlo_nz, arrs


def kernel(x, kernel):
    global last_exec_ns, last_results
    from concourse.bass_utils import run_bass_kernel_spmd

    x = np.ascontiguousarray(np.asarray(x, np.float32))
    w = np.asarray(kernel, np.float32)[::-1, ::-1]
    wlo_nz, warrs = _host_arrays(w)

    if wlo_nz not in _cache:
        _cache[wlo_nz] = _build(wlo_nz)
    nc = _cache[wlo_nz]

    # pack planes -> [pair, r, c, g, w] bf16 (plane = 2*pair+g, row = c*128+r)
    hi = x.reshape(_NCORES * _NPAIR, 2, 2, 128, _W).astype(_BF)  # [pair,g,c,r,w]
    xlk = np.ascontiguousarray(hi.transpose(0, 3, 2, 1, 4))

    in_maps = []
    for c in range(_NCORES):
        mp = {"xl": xlk[c * _NPAIR:(c + 1) * _NPAIR]}
        mp.update(warrs)
        in_maps.append(mp)

    trace = bool(os.environ.get("BLUR_TRACE"))
    tmpdir = os.environ.get("BLUR_TRACE_DIR") or None
    if trace:
        try:
            res = run_bass_kernel_spmd(nc, in_maps, list(range(_NCORES)),
                                       trace=True, tmpdir=tmpdir)
            last_exec_ns = res.exec_time_ns
        except Exception as e:
            print(f"trace run failed ({type(e).__name__}: {e}); retrying untraced")
            res = run_bass_kernel_spmd(nc, in_maps, list(range(_NCORES)))
            last_exec_ns = None
    else:
        res = run_bass_kernel_spmd(nc, in_maps, list(range(_NCORES)))
        last_exec_ns = None
    last_results = res

    full = np.empty((_NCORES * _PL, _OW, _OP), dtype=_BF)
    for c in range(_NCORES):
        o = res.results[c]["out"]      # [8, 127, 4, 2, 2, 512] bf16
        sm = res.results[c]["outs"]    # [96, 512] bf16
        main = np.asarray(o).transpose(0, 2, 3, 1, 4, 5).reshape(_PL, 2, 254, _OP)
        blk = full[c * _PL:(c + 1) * _PL]
        blk[:, 0:254] = main[:, 0]
        blk[:, 257:511] = main[:, 1]
        blk[:, 254:257] = np.asarray(sm).reshape(_PL, 3, _OP)
    return full[:, :, :_OW].reshape(4, 64, _OW, _OW).astype(np.float32)


# revision 21
# speedup vs baseline: 1.1983x; 1.1983x over previous
"""StyleGAN2 up-2x blur (upfirdn2d, up=2, pad=(2,1), 4x4 kernel) on 8 trn2 cores.

x: (4, 64, 256, 256) f32, kernel: (4, 4) f32 -> out: (4, 64, 511, 511) f32.

Polyphase decomposition: out[2r+s, 2c+t] is a 2x2-tap conv of x with weights
from the flipped kernel w = kernel[::-1, ::-1]:
  s=0 -> vertical taps (w[0,kx] @ r-1, w[2,kx] @ r); s=1 -> (w[1,kx] @ r, w[3,kx] @ r+1)
  t=0 -> horizontal taps kx in {0 (c-1), 2 (c)};    t=1 -> kx in {1 (c), 3 (c+1)}

Sharding: pure data parallel over the 256 (N*C) planes, 32 planes/core.

Numerics: the 2e-2 tolerance allows bf16 end-to-end - x is rounded to bf16
host-side (the [1,3,3,1] filter is exactly bf16-representable; a w_lo
correction path exists for general kernels), matmuls accumulate in fp32
PSUM, the output is stored as bf16 and upconverted on the host (rel err
~5e-3).  This halves both load and store HBM traffic.

Perf model (hard-won from traces):
- SWDGE stores are paced by Q7 descriptor emission (~10ns/desc, serial) and
  EACH SWDGE OP DRAINS ON A SINGLE SDMA ENGINE (~27 GB/s with 32KB packets).
  So the output is stored as an SBUF-LINEAR dump (the host unscrambles the
  layout for free): 16KB contiguous descriptors, 512 of them total, split
  into 64 ops of [16 partitions, 16KB] so consecutive ops round-robin onto
  all 16 engines.  Aggregate store rate then sits at the HBM write ceiling
  (~215 GB/s).  HWDGE stores are 10x worse (dispatch serializes ~10.5us/op
  on the issuing engine); scatter-to-NCHW layouts cost 8128 descriptors of
  Q7 emission; 4-d APs hit a slower software emission path - all dead ends.
- The whole output (131KB/partition) is buffered in SBUF (mpool bufs=8), so
  TensorE never stalls on stores: PE micro-idle would re-throttle the HAM
  clock (k=8 -> k=4) and double matmul time.
- The vertical 2-tap combine runs on TensorE as banded [128,128] stationary
  matmuls; horizontal taps are column-shifted moving operands accumulating
  into the same PSUM bank; 2 planes packed per matmul (free = 512).
- Output rows 254/255/256 straddle the two 128-row input chunks; they are
  computed for all 32 planes at once with gather stationaries over a
  [96, 256] tile holding plane rows 126/127/128 (4 matmuls total).
- Input is host-packed as [pair, r, c, g, w] so each load is one 256KB
  HWDGE op with 2KB contiguous per-partition chunks (HWDGE ops do spread
  across all 16 engines), and chunk-c moving slices stay contiguous.
"""

import os
import numpy as np
import ml_dtypes

_BF = ml_dtypes.bfloat16
_NCORES = 8
_PL = 32            # planes per core
_NPAIR = _PL // 2   # plane pairs per core
_MEGA = 2           # plane pairs per mega store tile
_NMEGA = _NPAIR // _MEGA
_H = 256
_W = 256
_OW = 511
_OP = 512           # padded output row pitch (host crops col 511)

_cache = {}
last_exec_ns = None
last_results = None


def _build(wlo_nz: bool):
    from contextlib import ExitStack
    import concourse.mybir as mybir
    import concourse.tile as tile
    from concourse import bacc

    BF = mybir.dt.bfloat16
    F32 = mybir.dt.float32

    nc = bacc.Bacc("TRN2", target_bir_lowering=False, debug=False)
    # input planes, host-packed: [pair, r, c(row chunk), g(plane in pair), w]
    # with plane row = c*128 + r
    xl = nc.dram_tensor("xl", [_NPAIR, 128, 2, 2, _W], BF, kind="ExternalInput").ap()
    sth = nc.dram_tensor("sth", [128, 12, 128], BF, kind="ExternalInput").ap()
    sgh = nc.dram_tensor("sgh", [128, 4, 128], BF, kind="ExternalInput").ap()
    if wlo_nz:
        stl = nc.dram_tensor("stl", [128, 12, 128], BF, kind="ExternalInput").ap()
        sgl = nc.dram_tensor("sgl", [128, 4, 128], BF, kind="ExternalInput").ap()
    # device output is SBUF-linear ("scrambled"); the host unscrambles.
    # out[mega, p, pl, cb, slot, w]: plane mega*2*_MEGA+pl, row-chunk cb,
    # out row (0 if cb==0 else 257) + 2p + slot, col w (pitch 512, 511=pad)
    out = nc.dram_tensor("out", [_NMEGA, 127, 2 * _MEGA, 2, 2, _OP],
                         BF, kind="ExternalOutput").ap()
    # seam rows: outs[3*plane + bi, w] = out row 254+bi of plane
    outs = nc.dram_tensor("outs", [96, _OP], BF, kind="ExternalOutput").ap()

    ncopy = 0  # alternate evacuation copies between VectorE and ScalarE

    with tile.TileContext(nc) as tc, ExitStack() as ctx:
        cpool = ctx.enter_context(tc.tile_pool(name="const", bufs=1))
        tpool = ctx.enter_context(tc.tile_pool(name="tin", bufs=4))
        epool = ctx.enter_context(tc.tile_pool(name="edge", bufs=1))
        mpool = ctx.enter_context(tc.tile_pool(name="mega", bufs=_NMEGA))
        bpool = ctx.enter_context(tc.tile_pool(name="bnd", bufs=1))
        ppool = ctx.enter_context(tc.tile_pool(name="ps", bufs=8, space="PSUM"))

        sth_t = cpool.tile([128, 12, 128], BF)
        nc.sync.dma_start(out=sth_t[:, :, :], in_=sth)
        sgh_t = cpool.tile([128, 4, 128], BF)
        nc.sync.dma_start(out=sgh_t[:, :, :], in_=sgh)
        if wlo_nz:
            stl_t = cpool.tile([128, 12, 128], BF)
            nc.sync.dma_start(out=stl_t[:, :, :], in_=stl)
            sgl_t = cpool.tile([128, 4, 128], BF)
            nc.sync.dma_start(out=sgl_t[:, :, :], in_=sgl)

        def copy_out(dst, src):
            nonlocal ncopy
            if ncopy % 2 == 0:
                nc.vector.tensor_copy(out=dst, in_=src)
            else:
                nc.scalar.copy(out=dst, in_=src)
            ncopy += 1

        # ---- seam rows oy=254 (s0,r=127: x[126],x[127]), oy=255 (s1,r=127:
        # ---- x[127],x[128]), oy=256 (s0,r=128: x[127],x[128]) for all 32
        # planes at once: e holds plane rows 126/127/128 at partition
        # 32*rk + 16*g + pair; gather stationaries sgh[:, kx, :] map them to
        # psum partition q = 3*plane + bi with the vertical taps folded in.
        # Emitted after mega 0 so the main pipeline starts immediately.
        def emit_seam():
            e = epool.tile([96, _W], BF)
            for rk, row in enumerate((126, 127, 128)):
                rr, cc = row % 128, row // 128
                for g in (0, 1):
                    nc.sync.dma_start(out=e[32 * rk + 16 * g:32 * rk + 16 * g + 16, :],
                                      in_=xl[:, rr, cc, g, :])

            bt = bpool.tile([96, _OP], BF)
            pb = ppool.tile([96, 2, 256], F32, tag="ps")
            mms = []
            for t_, kx, mv, pc in ((0, 2, (0, 256), (0, 256)), (0, 0, (0, 255), (1, 256)),
                                   (1, 1, (0, 256), (0, 256)), (1, 3, (1, 256), (0, 255))):
                mms.append((t_, kx, mv, pc, "h"))
                if wlo_nz:
                    mms.append((t_, kx, mv, pc, "l"))
            for i, (t_, kx, mv, pc, wp) in enumerate(mms):
                sg = sgh_t if wp == "h" else sgl_t
                nc.tensor.matmul(
                    pb[:, t_, pc[0]:pc[1]], sg[0:96, kx, 0:96], e[:, mv[0]:mv[1]],
                    start=(i == 0), stop=(i == len(mms) - 1))
            copy_out(bt[:, 0:_OP:2], pb[:, 0, :])
            copy_out(bt[:, 1:_OP:2], pb[:, 1, :])
            nc.gpsimd.dma_start(out=outs, in_=bt[:, :])

        # ---- main body: 4 megas x 4 pairs x 2 row-chunks
        # stationary groups: 0 = s0/chunkA (rows 0..126), 1 = s0/chunkB, 2 = s1
        for mega in range(_NMEGA):
            # mega assembly tile: [p, plane_in_mega, chunk, rowpair slot, col]
            m = mpool.tile([128, 2 * _MEGA, 2, 2, _OP], BF, tag="m")
            for pp in range(_MEGA):
                pair = mega * _MEGA + pp
                t = tpool.tile([128, 2, 2, _W], BF, tag="tin")
                nc.sync.dma_start(
                    out=t[:, :, :, :].rearrange("r c g w -> r (c g w)"),
                    in_=xl[pair].rearrange("r c g w -> r (c g w)"))
                for chunk in (0, 1):
                    ig0 = 0 if chunk == 0 else 1
                    # chunk A row-pair layout: [i,0]=s0A[i] (oy 2i),
                    # [i,1]=s1A[i] (oy 2i+1); chunk B: [i,0]=s1B[i]
                    # (oy 257+2i), [i,1]=s0B[i] (oy 258+2i)
                    rows = ((0, 1) if chunk == 0 else (1, 0))  # s feeding (slot0, slot1)
                    # 2-plane-packed moving/psum as FLAT 1-d APs (flat index
                    # = 256*plane + col): a 2-d free AP costs the PE a ~200
                    # cycle bubble per outer step (measured 379ns vs 213ns
                    # for N=512).  The plane-crossing element of the
                    # column-shifted kx=3 tap lands in psum col 255 = the
                    # cropped pad column, so it's harmless; kx=0 (shift the
                    # other way) would pollute plane 1's col 0, so it keeps
                    # the 2-d AP.
                    tv = t[:, chunk, :, :].rearrange("r g w -> r (g w)")
                    for s, ig in ((0, ig0), (1, 2)):
                        for t_, kxmv in ((0, ((2, "1d", (0, 512), (0, 512)), (0, "2d", (0, 255), (1, 256)))),
                                         (1, ((1, "1d", (0, 512), (0, 512)), (3, "1d", (1, 512), (0, 511))))):
                            pt = ppool.tile([128, 2, 256], F32, tag="ps")
                            pv = pt[:, :, :].rearrange("p g w -> p (g w)")
                            mms = []
                            for kx, md, mv, pc in kxmv:
                                mms.append((ig * 4 + kx, md, mv, pc, "h"))
                                if wlo_nz:
                                    mms.append((ig * 4 + kx, md, mv, pc, "l"))
                            for i, (j, md, mv, pc, wp) in enumerate(mms):
                                st_ = sth_t if wp == "h" else stl_t
                                if md == "1d":
                                    nc.tensor.matmul(
                                        pv[:, pc[0]:pc[1]], st_[:, j, :],
                                        tv[:, mv[0]:mv[1]],
                                        start=(i == 0), stop=(i == len(mms) - 1))
                                else:
                                    nc.tensor.matmul(
                                        pt[:, :, pc[0]:pc[1]], st_[:, j, :],
                                        t[:, chunk, :, mv[0]:mv[1]],
                                        start=(i == 0), stop=(i == len(mms) - 1))
                            # drain this psum group immediately (overlaps with
                            # the next group's matmuls), casting f32 -> bf16;
                            # both planes in one op (free dims [2, 256])
                            slot = rows.index(s)
                            dst = m[0:127, 2 * pp:2 * pp + 2, chunk, slot, :]
                            off = 0 if t_ == 0 else 1
                            copy_out(dst[:, :, off:_OP:2], pt[0:127, :, :])

            # store: SWDGE ops striped across SDMA engines (one op drains
            # on ONE engine; saturated per-engine rate is ~13-16GB/s for any
            # packet size >= 16KB).  Normal megas: 8 ops x [16 parts, 16KB
            # descs]; the LAST mega is split 16 ways so the drain tail runs
            # at full 16-engine width instead of half.
            nsplit = 16 if mega == _NMEGA - 1 else 8
            for e_ in range(nsplit):
                w_ = 128 // nsplit
                p0, p1 = w_ * e_, min(w_ * e_ + w_, 127)
                nc.gpsimd.dma_start(
                    out=out[mega, p0:p1].rearrange("p a b c w -> p (a b c w)"),
                    in_=m[p0:p1, :, :, :, :].rearrange("p a b c w -> p (a b c w)"))
            if mega == 0:
                emit_seam()

    nc.compile()
    return nc


def _host_arrays(w):
    w = np.asarray(w, np.float32)
    w_hi = w.astype(_BF).astype(np.float32)
    w_lo = w - w_hi
    wlo_nz = bool(np.any(w_lo != 0))

    def build_st(wv):
        st = np.zeros((3, 4, 128, 128), np.float32)
        i6 = np.arange(126)
        i7 = np.arange(127)
        for kx in range(4):
            st[0, kx][i6, i6 + 1] = wv[0, kx]        # s0A subdiag, out rows 1..126
            st[0, kx][i7, i7] += wv[2, kx]           # s0A diag, out rows 0..126
            st[1, kx][i7, i7] = wv[0, kx]            # s0B diag
            st[1, kx][i7 + 1, i7] = wv[2, kx]        # s0B sub
            st[2, kx][i7, i7] = wv[1, kx]            # s1 diag
            st[2, kx][i7 + 1, i7] = wv[3, kx]        # s1 sub
        # [g,kx,p,i] -> [p, g*4+kx, i]
        return np.ascontiguousarray(
            st.reshape(12, 128, 128).transpose(1, 0, 2)).astype(_BF)

    def build_sg(wv):
        # seam gather: partition p = 32*rk + 16*g + pair holds plane row
        # (126,127,128)[rk] of plane 2*pair+g; output q = 3*plane + bi
        # (bi: seam row 254+bi); vertical taps per bi: (rk, ky) pairs
        taps = {0: ((0, 0), (1, 2)), 1: ((1, 1), (2, 3)), 2: ((1, 0), (2, 2))}
        sg = np.zeros((4, 128, 128), np.float32)
        for kx in range(4):
            for pair in range(16):
                for g in (0, 1):
                    plane = 2 * pair + g
                    for bi, tl in taps.items():
                        for rk, ky in tl:
                            sg[kx][32 * rk + 16 * g + pair, 3 * plane + bi] += wv[ky, kx]
        return np.ascontiguousarray(sg.transpose(1, 0, 2)).astype(_BF)  # [p, kx, q]

    arrs = {"sth": build_st(w_hi), "sgh": build_sg(w_hi)}
    if wlo_nz:
        wlo_b = w_lo.astype(_BF).astype(np.float32)
        arrs["stl"] = build_st(wlo_b)
        arrs["sgl"] = build_sg(wlo_b)
    return wlo_nz, arrs


def kernel(x, kernel):
    global last_exec_ns, last_results
    from concourse.bass_utils import run_bass_kernel_spmd

    x = np.ascontiguousarray(np.asarray(x, np.float32))
    w = np.asarray(kernel, np.float32)[::-1, ::-1]
    wlo_nz, warrs = _host_arrays(w)

    if wlo_nz not in _cache:
        _cache[wlo_nz] = _build(wlo_nz)
    nc = _cache[wlo_nz]

    # pack planes -> [pair, r, c, g, w] bf16 (plane = 2*pair+g, row = c*128+r)
    hi = x.reshape(_NCORES * _NPAIR, 2, 2, 128, _W).astype(_BF)  # [pair,g,c,r,w]
    xlk = np.ascontiguousarray(hi.transpose(0, 3, 2, 1, 4))

    in_maps = []
    for c in range(_NCORES):
        mp = {"xl": xlk[c * _NPAIR:(c + 1) * _NPAIR]}
        mp.update(warrs)
        in_maps.append(mp)

    trace = bool(os.environ.get("BLUR_TRACE"))
    tmpdir = os.environ.get("BLUR_TRACE_DIR") or None
    if trace:
        try:
            res = run_bass_kernel_spmd(nc, in_maps, list(range(_NCORES)),
                                       trace=True, tmpdir=tmpdir)
            last_exec_ns = res.exec_time_ns
        except Exception as e:
            print(f"trace run failed ({type(e).__name__}: {e}); retrying untraced")
            res = run_bass_kernel_spmd(nc, in_maps, list(range(_NCORES)))
            last_exec_ns = None
    else:
        res = run_bass_kernel_spmd(nc, in_maps, list(range(_NCORES)))
        last_exec_ns = None
    last_results = res

    full = np.empty((_NCORES * _PL, _OW, _OP), dtype=_BF)
    for c in range(_NCORES):
        o = res.results[c]["out"]      # [8, 127, 4, 2, 2, 512] bf16
        sm = res.results[c]["outs"]    # [96, 512] bf16
        main = np.asarray(o).transpose(0, 2, 3, 1, 4, 5).reshape(_PL, 2, 254, _OP)
        blk = full[c * _PL:(c + 1) * _PL]
        blk[:, 0:254] = main[:, 0]
        blk[:, 257:511] = main[:, 1]
        blk[:, 254:257] = np.asarray(sm).reshape(_PL, 3, _OP)
    return full[:, :, :_OW].reshape(4, 64, _OW, _OW).astype(np.float32)


# revision 23
# speedup vs baseline: 1.2483x; 1.0417x over previous
"""StyleGAN2 up-2x blur (upfirdn2d, up=2, pad=(2,1), 4x4 kernel) on 8 trn2 cores.

x: (4, 64, 256, 256) f32, kernel: (4, 4) f32 -> out: (4, 64, 511, 511) f32.

Polyphase decomposition: out[2r+s, 2c+t] is a 2x2-tap conv of x with weights
from the flipped kernel w = kernel[::-1, ::-1]:
  s=0 -> vertical taps (w[0,kx] @ r-1, w[2,kx] @ r); s=1 -> (w[1,kx] @ r, w[3,kx] @ r+1)
  t=0 -> horizontal taps kx in {0 (c-1), 2 (c)};    t=1 -> kx in {1 (c), 3 (c+1)}

Sharding: pure data parallel over the 256 (N*C) planes, 32 planes/core.

Numerics: the 2e-2 tolerance allows bf16 end-to-end - x is rounded to bf16
host-side (the [1,3,3,1] filter is exactly bf16-representable; a w_lo
correction path exists for general kernels), matmuls accumulate in fp32
PSUM, the output is stored as bf16 and upconverted on the host (rel err
~5e-3).  This halves both load and store HBM traffic.

Perf model (hard-won from traces):
- SWDGE stores are paced by Q7 descriptor emission (~10ns/desc, serial) and
  EACH SWDGE OP DRAINS ON A SINGLE SDMA ENGINE (~27 GB/s with 32KB packets).
  So the output is stored as an SBUF-LINEAR dump (the host unscrambles the
  layout for free): 16KB contiguous descriptors, 512 of them total, split
  into 64 ops of [16 partitions, 16KB] so consecutive ops round-robin onto
  all 16 engines.  Aggregate store rate then sits at the HBM write ceiling
  (~215 GB/s).  HWDGE stores are 10x worse (dispatch serializes ~10.5us/op
  on the issuing engine); scatter-to-NCHW layouts cost 8128 descriptors of
  Q7 emission; 4-d APs hit a slower software emission path - all dead ends.
- The whole output (131KB/partition) is buffered in SBUF (mpool bufs=8), so
  TensorE never stalls on stores: PE micro-idle would re-throttle the HAM
  clock (k=8 -> k=4) and double matmul time.
- The vertical 2-tap combine runs on TensorE as banded [128,128] stationary
  matmuls; horizontal taps are column-shifted moving operands accumulating
  into the same PSUM bank; 2 planes packed per matmul (free = 512).
- Output rows 254/255/256 straddle the two 128-row input chunks; they are
  computed for all 32 planes at once with gather stationaries over a
  [96, 256] tile holding plane rows 126/127/128 (4 matmuls total).
- Input is host-packed as [pair, r, c, g, w] so each load is one 256KB
  HWDGE op with 2KB contiguous per-partition chunks (HWDGE ops do spread
  across all 16 engines), and chunk-c moving slices stay contiguous.
"""

import os
import numpy as np
import ml_dtypes

_BF = ml_dtypes.bfloat16
_NCORES = 8
_PL = 32            # planes per core
_NPAIR = _PL // 2   # plane pairs per core
_MEGA = 2           # plane pairs per mega store tile
_NMEGA = _NPAIR // _MEGA
_H = 256
_W = 256
_OW = 511
_OP = 512           # padded output row pitch (host crops col 511)

_cache = {}
last_exec_ns = None
last_results = None


def _build(wlo_nz: bool):
    from contextlib import ExitStack
    import concourse.mybir as mybir
    import concourse.tile as tile
    from concourse import bacc

    BF = mybir.dt.bfloat16
    F32 = mybir.dt.float32

    nc = bacc.Bacc("TRN2", target_bir_lowering=False, debug=False)
    # input planes, host-packed: [pair, r, c(row chunk), g(plane in pair), w]
    # with plane row = c*128 + r
    xl = nc.dram_tensor("xl", [_NPAIR, 128, 2, 2, _W], BF, kind="ExternalInput").ap()
    sth = nc.dram_tensor("sth", [128, 12, 128], BF, kind="ExternalInput").ap()
    sgh = nc.dram_tensor("sgh", [128, 4, 128], BF, kind="ExternalInput").ap()
    if wlo_nz:
        stl = nc.dram_tensor("stl", [128, 12, 128], BF, kind="ExternalInput").ap()
        sgl = nc.dram_tensor("sgl", [128, 4, 128], BF, kind="ExternalInput").ap()
    # device output is SBUF-linear ("scrambled"); the host unscrambles.
    # out[mega, p, pl, cb, slot, w]: plane mega*2*_MEGA+pl, row-chunk cb,
    # out row (0 if cb==0 else 257) + 2p + slot, col w (pitch 512, 511=pad)
    out = nc.dram_tensor("out", [_NMEGA, 127, 2 * _MEGA, 2, 2, _OP],
                         BF, kind="ExternalOutput").ap()
    # seam rows: outs[3*plane + bi, w] = out row 254+bi of plane
    outs = nc.dram_tensor("outs", [96, _OP], BF, kind="ExternalOutput").ap()

    ncopy = 0  # alternate evacuation copies between VectorE and ScalarE

    with tile.TileContext(nc) as tc, ExitStack() as ctx:
        cpool = ctx.enter_context(tc.tile_pool(name="const", bufs=1))
        tpool = ctx.enter_context(tc.tile_pool(name="tin", bufs=8))
        epool = ctx.enter_context(tc.tile_pool(name="edge", bufs=1))
        mpool = ctx.enter_context(tc.tile_pool(name="mega", bufs=_NMEGA))
        bpool = ctx.enter_context(tc.tile_pool(name="bnd", bufs=1))
        ppool = ctx.enter_context(tc.tile_pool(name="ps", bufs=8, space="PSUM"))

        sth_t = cpool.tile([128, 12, 128], BF)
        nc.sync.dma_start(out=sth_t[:, :, :], in_=sth)
        sgh_t = cpool.tile([128, 4, 128], BF)
        nc.sync.dma_start(out=sgh_t[:, :, :], in_=sgh)
        if wlo_nz:
            stl_t = cpool.tile([128, 12, 128], BF)
            nc.sync.dma_start(out=stl_t[:, :, :], in_=stl)
            sgl_t = cpool.tile([128, 4, 128], BF)
            nc.sync.dma_start(out=sgl_t[:, :, :], in_=sgl)

        def copy_out(dst, src):
            nonlocal ncopy
            if ncopy % 2 == 0:
                nc.vector.tensor_copy(out=dst, in_=src)
            else:
                nc.scalar.copy(out=dst, in_=src)
            ncopy += 1

        # ---- seam rows oy=254 (s0,r=127: x[126],x[127]), oy=255 (s1,r=127:
        # ---- x[127],x[128]), oy=256 (s0,r=128: x[127],x[128]) for all 32
        # planes at once: e holds plane rows 126/127/128 at partition
        # 32*rk + 16*g + pair; gather stationaries sgh[:, kx, :] map them to
        # psum partition q = 3*plane + bi with the vertical taps folded in.
        # Emitted after mega 0 so the main pipeline starts immediately.
        def emit_seam():
            e = epool.tile([96, _W], BF)
            for rk, row in enumerate((126, 127, 128)):
                rr, cc = row % 128, row // 128
                for g in (0, 1):
                    nc.sync.dma_start(out=e[32 * rk + 16 * g:32 * rk + 16 * g + 16, :],
                                      in_=xl[:, rr, cc, g, :])

            bt = bpool.tile([96, _OP], BF)
            pb = ppool.tile([96, 2, 256], F32, tag="ps")
            mms = []
            for t_, kx, mv, pc in ((0, 2, (0, 256), (0, 256)), (0, 0, (0, 255), (1, 256)),
                                   (1, 1, (0, 256), (0, 256)), (1, 3, (1, 256), (0, 255))):
                mms.append((t_, kx, mv, pc, "h"))
                if wlo_nz:
                    mms.append((t_, kx, mv, pc, "l"))
            for i, (t_, kx, mv, pc, wp) in enumerate(mms):
                sg = sgh_t if wp == "h" else sgl_t
                nc.tensor.matmul(
                    pb[:, t_, pc[0]:pc[1]], sg[0:96, kx, 0:96], e[:, mv[0]:mv[1]],
                    start=(i == 0), stop=(i == len(mms) - 1))
            copy_out(bt[:, 0:_OP:2], pb[:, 0, :])
            copy_out(bt[:, 1:_OP:2], pb[:, 1, :])
            nc.gpsimd.dma_start(out=outs, in_=bt[:, :])

        # ---- main body: 8 megas x 2 pairs x 2 row-chunks
        # stationary groups: 0 = s0/chunkA (rows 0..126), 1 = s0/chunkB, 2 = s1
        # all input loads are issued up front (2 pairs per op, 4KB descs) so
        # they finish early and stop competing with stores for SDMA engine
        # slots and DMA completion-semaphore lanes
        ttiles = []
        for q in range(_NPAIR // 2):
            t = tpool.tile([128, 2, 2, 2, _W], BF, tag="tin")
            nc.sync.dma_start(
                out=t[:, :, :, :, :].rearrange("r p c g w -> r p (c g w)"),
                in_=xl[2 * q:2 * q + 2].rearrange("p r c g w -> r p (c g w)"))
            ttiles.append(t)
        for mega in range(_NMEGA):
            # mega assembly tile: [p, plane_in_mega, chunk, rowpair slot, col]
            m = mpool.tile([128, 2 * _MEGA, 2, 2, _OP], BF, tag="m")
            for pp in range(_MEGA):
                pair = mega * _MEGA + pp
                t = ttiles[pair // 2][:, pair % 2]
                for chunk in (0, 1):
                    ig0 = 0 if chunk == 0 else 1
                    # chunk A row-pair layout: [i,0]=s0A[i] (oy 2i),
                    # [i,1]=s1A[i] (oy 2i+1); chunk B: [i,0]=s1B[i]
                    # (oy 257+2i), [i,1]=s0B[i] (oy 258+2i)
                    rows = ((0, 1) if chunk == 0 else (1, 0))  # s feeding (slot0, slot1)
                    # 2-plane-packed moving/psum as FLAT 1-d APs (flat index
                    # = 256*plane + col): a 2-d free AP costs the PE a ~200
                    # cycle bubble per outer step (measured 379ns vs 213ns
                    # for N=512).  The plane-crossing element of the
                    # column-shifted kx=3 tap lands in psum col 255 = the
                    # cropped pad column, so it's harmless; kx=0 (shift the
                    # other way) would pollute plane 1's col 0, so it keeps
                    # the 2-d AP.
                    tv = t[:, chunk, :, :].rearrange("r g w -> r (g w)")
                    for s, ig in ((0, ig0), (1, 2)):
                        for t_, kxmv in ((0, ((2, "1d", (0, 512), (0, 512)), (0, "2d", (0, 255), (1, 256)))),
                                         (1, ((1, "1d", (0, 512), (0, 512)), (3, "1d", (1, 512), (0, 511))))):
                            pt = ppool.tile([128, 2, 256], F32, tag="ps")
                            pv = pt[:, :, :].rearrange("p g w -> p (g w)")
                            mms = []
                            for kx, md, mv, pc in kxmv:
                                mms.append((ig * 4 + kx, md, mv, pc, "h"))
                                if wlo_nz:
                                    mms.append((ig * 4 + kx, md, mv, pc, "l"))
                            for i, (j, md, mv, pc, wp) in enumerate(mms):
                                st_ = sth_t if wp == "h" else stl_t
                                if md == "1d":
                                    nc.tensor.matmul(
                                        pv[:, pc[0]:pc[1]], st_[:, j, :],
                                        tv[:, mv[0]:mv[1]],
                                        start=(i == 0), stop=(i == len(mms) - 1))
                                else:
                                    nc.tensor.matmul(
                                        pt[:, :, pc[0]:pc[1]], st_[:, j, :],
                                        t[:, chunk, :, mv[0]:mv[1]],
                                        start=(i == 0), stop=(i == len(mms) - 1))
                            # drain this psum group immediately (overlaps with
                            # the next group's matmuls), casting f32 -> bf16;
                            # both planes in one op (free dims [2, 256])
                            slot = rows.index(s)
                            dst = m[0:127, 2 * pp:2 * pp + 2, chunk, slot, :]
                            off = 0 if t_ == 0 else 1
                            copy_out(dst[:, :, off:_OP:2], pt[0:127, :, :])

            # store: SWDGE ops striped across SDMA engines (one op drains
            # on ONE engine; saturated per-engine rate is ~13-16GB/s for any
            # packet size >= 16KB).  Normal megas: 8 ops x [16 parts, 16KB
            # descs]; the LAST mega is split 16 ways so the drain tail runs
            # at full 16-engine width instead of half.
            nsplit = 16 if mega == _NMEGA - 1 else 8
            for e_ in range(nsplit):
                w_ = 128 // nsplit
                p0, p1 = w_ * e_, min(w_ * e_ + w_, 127)
                nc.gpsimd.dma_start(
                    out=out[mega, p0:p1].rearrange("p a b c w -> p (a b c w)"),
                    in_=m[p0:p1, :, :, :, :].rearrange("p a b c w -> p (a b c w)"))
            if mega == 0:
                emit_seam()

    nc.compile()
    return nc


def _host_arrays(w):
    w = np.asarray(w, np.float32)
    w_hi = w.astype(_BF).astype(np.float32)
    w_lo = w - w_hi
    wlo_nz = bool(np.any(w_lo != 0))

    def build_st(wv):
        st = np.zeros((3, 4, 128, 128), np.float32)
        i6 = np.arange(126)
        i7 = np.arange(127)
        for kx in range(4):
            st[0, kx][i6, i6 + 1] = wv[0, kx]        # s0A subdiag, out rows 1..126
            st[0, kx][i7, i7] += wv[2, kx]           # s0A diag, out rows 0..126
            st[1, kx][i7, i7] = wv[0, kx]            # s0B diag
            st[1, kx][i7 + 1, i7] = wv[2, kx]        # s0B sub
            st[2, kx][i7, i7] = wv[1, kx]            # s1 diag
            st[2, kx][i7 + 1, i7] = wv[3, kx]        # s1 sub
        # [g,kx,p,i] -> [p, g*4+kx, i]
        return np.ascontiguousarray(
            st.reshape(12, 128, 128).transpose(1, 0, 2)).astype(_BF)

    def build_sg(wv):
        # seam gather: partition p = 32*rk + 16*g + pair holds plane row
        # (126,127,128)[rk] of plane 2*pair+g; output q = 3*plane + bi
        # (bi: seam row 254+bi); vertical taps per bi: (rk, ky) pairs
        taps = {0: ((0, 0), (1, 2)), 1: ((1, 1), (2, 3)), 2: ((1, 0), (2, 2))}
        sg = np.zeros((4, 128, 128), np.float32)
        for kx in range(4):
            for pair in range(16):
                for g in (0, 1):
                    plane = 2 * pair + g
                    for bi, tl in taps.items():
                        for rk, ky in tl:
                            sg[kx][32 * rk + 16 * g + pair, 3 * plane + bi] += wv[ky, kx]
        return np.ascontiguousarray(sg.transpose(1, 0, 2)).astype(_BF)  # [p, kx, q]

    arrs = {"sth": build_st(w_hi), "sgh": build_sg(w_hi)}
    if wlo_nz:
        wlo_b = w_lo.astype(_BF).astype(np.float32)
        arrs["stl"] = build_st(wlo_b)
        arrs["sgl"] = build_sg(wlo_b)
    return wlo_nz, arrs


def kernel(x, kernel):
    global last_exec_ns, last_results
    from concourse.bass_utils import run_bass_kernel_spmd

    x = np.ascontiguousarray(np.asarray(x, np.float32))
    w = np.asarray(kernel, np.float32)[::-1, ::-1]
    wlo_nz, warrs = _host_arrays(w)

    if wlo_nz not in _cache:
        _cache[wlo_nz] = _build(wlo_nz)
    nc = _cache[wlo_nz]

    # pack planes -> [pair, r, c, g, w] bf16 (plane = 2*pair+g, row = c*128+r)
    hi = x.reshape(_NCORES * _NPAIR, 2, 2, 128, _W).astype(_BF)  # [pair,g,c,r,w]
    xlk = np.ascontiguousarray(hi.transpose(0, 3, 2, 1, 4))

    in_maps = []
    for c in range(_NCORES):
        mp = {"xl": xlk[c * _NPAIR:(c + 1) * _NPAIR]}
        mp.update(warrs)
        in_maps.append(mp)

    trace = bool(os.environ.get("BLUR_TRACE"))
    tmpdir = os.environ.get("BLUR_TRACE_DIR") or None
    if trace:
        try:
            res = run_bass_kernel_spmd(nc, in_maps, list(range(_NCORES)),
                                       trace=True, tmpdir=tmpdir)
            last_exec_ns = res.exec_time_ns
        except Exception as e:
            print(f"trace run failed ({type(e).__name__}: {e}); retrying untraced")
            res = run_bass_kernel_spmd(nc, in_maps, list(range(_NCORES)))
            last_exec_ns = None
    else:
        res = run_bass_kernel_spmd(nc, in_maps, list(range(_NCORES)))
        last_exec_ns = None
    last_results = res

    full = np.empty((_NCORES * _PL, _OW, _OP), dtype=_BF)
    for c in range(_NCORES):
        o = res.results[c]["out"]      # [8, 127, 4, 2, 2, 512] bf16
        sm = res.results[c]["outs"]    # [96, 512] bf16
        main = np.asarray(o).transpose(0, 2, 3, 1, 4, 5).reshape(_PL, 2, 254, _OP)
        blk = full[c * _PL:(c + 1) * _PL]
        blk[:, 0:254] = main[:, 0]
        blk[:, 257:511] = main[:, 1]
        blk[:, 254:257] = np.asarray(sm).reshape(_PL, 3, _OP)
    return full[:, :, :_OW].reshape(4, 64, _OW, _OW).astype(np.float32)
